# revision 1
# baseline (speedup 1.0000x reference)
"""LightGCN-style GNN message passing on 8 Trainium2 NeuronCores.

Algorithm (matches the reference):
    deg  = bincount(dst);  dinv = rsqrt(max(deg, 1))
    x_{l+1} = dinv * (A @ (dinv * x_l))          (3 layers, A = binary adjacency)
    z_l = l2_normalize(x_l);  Z = concat(z_0..z_3);  Y = Z @ W.T + b
    return Y[senders], Y[receivers]

Key factorization: with xs_l = dinv * x_l, messages need no per-edge scale
(norm = dinv[src]*dinv[dst] splits into a pre-scale of the gathered table and
a post-scale of the scattered rows), and l2_normalize(xs_l) == l2_normalize(x_l)
since dinv > 0. So only the xs tables are ever materialized (bf16).

Sharding: destination-sharded. Core i owns N/8 dst rows; its edges are grouped
by (src-chunk, dst-block-of-128). Per 128-edge tile:
  - dma_gather 128 rows of xs (bf16, 256B rows) from the chunk window of the
    replicated table (int16 gather indices => 4 windows),
  - one-hot S[p, m] = (dst_local[p] == m) built on DVE via iota + is_equal,
  - PE matmul psum[block] += S.T @ msgs accumulates the segment sum on-chip.
Post-scale by dinv^2 on ACT -> bf16 own slice -> AllGather -> next layer.
Final: gather the 4 xs tables at the output rows this core handles,
l2-normalize, PE-transpose, matmul with W^T (bf16), add bias, DMA out.
The host only computes degrees and integer index/schedule arrays; all FLOAT
work on emb/W/b happens on device.
"""

import numpy as np
import ml_dtypes

import concourse.bacc as bacc
import concourse.mybir as mybir
import concourse.tile as tile

F32 = mybir.dt.float32
BF16 = mybir.dt.bfloat16
I16 = mybir.dt.int16
I32 = mybir.dt.int32

D = 128             # feature dim
NL = 3              # message passing layers
NC = 8              # cores
BLK = 128           # dst block (psum partition dim)
NCH = 4             # source chunks (int16 gather index limit)


class Cfg:
    def __init__(self, N, E, NOUT, GCALL=4096, SLAB=14):
        self.N = N
        self.E = E
        self.NOUT = NOUT
        self.GCALL = GCALL
        self.PER = N // NC
        self.NB = (self.PER + BLK - 1) // BLK
        self.SEG = self.NB * BLK
        self.NTOT = NC * self.SEG
        assert self.NTOT % NCH == 0
        self.CHUNK = self.NTOT // NCH
        assert self.CHUNK <= 32768, "int16 gather index overflow"
        assert self.CHUNK % 16 == 0
        self.OPC = NOUT // NC
        self.SLAB = min(SLAB, self.NB)


FULL = Cfg(N=100000, E=1600000, NOUT=16384)


def _ceil(a, b):
    return (a + b - 1) // b


def _wrap16(idx):
    """int16 [L] -> [128, L//16] wrapped in 16 partitions, replicated x8."""
    return np.tile(idx.reshape(-1, 16).T, (8, 1)).copy()


def _prep(cfg, emb, edge_index, senders, receivers, W, b):
    N, E, PER, SEG, NB, CHUNK = cfg.N, cfg.E, cfg.PER, cfg.SEG, cfg.NB, cfg.CHUNK
    src = np.asarray(edge_index[0], np.int64)
    dst = np.asarray(edge_index[1], np.int64)
    senders = np.asarray(senders, np.int64)
    receivers = np.asarray(receivers, np.int64)

    deg = np.bincount(dst, minlength=N).astype(np.float32)
    deg = np.maximum(deg, 1.0)
    dinv = (1.0 / np.sqrt(deg)).astype(np.float32)
    dinv2 = (dinv * dinv).astype(np.float32)

    # --- edge schedule -----------------------------------------------------
    core_of = dst // PER
    r = dst % PER
    blk = r // BLK
    dloc = r % BLK
    spn = (src // PER) * SEG + (src % PER)       # padded row of the source
    ch = spn // CHUNK
    sidx = (spn - ch * CHUNK).astype(np.int16)   # in-window gather index

    cell = (core_of * NCH + ch) * NB + blk       # [E] global (core,chunk,block)
    ncell = NC * NCH * NB
    counts = np.bincount(cell, minlength=ncell)
    T_cb = _ceil(counts.reshape(NC, NCH, NB), BLK).max(axis=0)  # [NCH, NB]

    tt_c = T_cb.sum(axis=1)                      # tiles per chunk stream
    ctile_off = np.concatenate([[0], np.cumsum(tt_c)[:-1]])
    TT = int(tt_c.sum())                         # total tiles per core
    TOTLEN = TT * BLK

    # tile base of each (c,b) within the concatenated per-core stream
    tb = np.zeros((NCH, NB), np.int64)
    for c in range(NCH):
        tb[c] = ctile_off[c] + np.concatenate([[0], np.cumsum(T_cb[c])[:-1]])
    base_cb = (tb * BLK).reshape(-1)             # [NCH*NB] edge-position base

    order = np.argsort(cell, kind="stable")
    cell_sorted = cell[order]
    starts = np.concatenate([[0], np.cumsum(counts)[:-1]])
    rank = np.arange(E, dtype=np.int64) - starts[cell_sorted]
    # cell % (NCH*NB) is the (chunk, block) flat index
    epos = base_cb[cell_sorted % (NCH * NB)] + rank

    eidx_arrs, edloc_arrs = [], []
    src_sorted = sidx[order]
    dloc_sorted = dloc[order]
    core_sorted = core_of[order]
    for i in range(NC):
        m = core_sorted == i
        ia = np.zeros(TOTLEN, np.int16)
        da = np.full(TOTLEN, -1.0, np.float32)
        ia[epos[m]] = src_sorted[m]
        da[epos[m]] = dloc_sorted[m]
        eidx_arrs.append(_wrap16(ia))                      # [128, TOTLEN//16]
        edloc_arrs.append(da.reshape(TT, BLK).T.copy())    # [128, TT]

    # --- output-row schedule ----------------------------------------------
    OPC = cfg.OPC
    ids = [np.concatenate([senders[OPC * i:OPC * (i + 1)],
                           receivers[OPC * i:OPC * (i + 1)]]) for i in range(NC)]
    fpn = [(x // PER) * SEG + (x % PER) for x in ids]
    fch = [x // CHUNK for x in fpn]
    gcnt = np.array([[int((fch[i] == c).sum()) for c in range(NCH)]
                     for i in range(NC)])
    OUT_T_c = _ceil(gcnt, BLK).max(axis=0)       # [NCH] tiles, shared
    OUT_T = int(OUT_T_c.sum())
    OUTLEN = OUT_T * BLK
    fbase = np.concatenate([[0], np.cumsum(OUT_T_c)[:-1]]) * BLK

    fidx_arrs, pos_arrs = [], []
    for i in range(NC):
        fo = np.argsort(fch[i], kind="stable")
        pos = np.zeros(2 * OPC, np.int64)
        ia = np.zeros(OUTLEN, np.int16)
        for c in range(NCH):
            m = fo[fch[i][fo] == c]
            p = fbase[c] + np.arange(len(m))
            pos[m] = p
            ia[p] = (fpn[i][m] - c * CHUNK).astype(np.int16)
        fidx_arrs.append(_wrap16(ia))            # [128, OUTLEN//16]
        pos_arrs.append(pos)

    # --- per-core dense inputs --------------------------------------------
    in_maps = []
    for i in range(NC):
        eo = np.zeros((SEG, D), np.float32)
        eo[:PER] = emb[PER * i:PER * (i + 1)]
        dv = np.zeros(SEG, np.float32)
        dv[:PER] = dinv[PER * i:PER * (i + 1)]
        dv2 = np.zeros(SEG, np.float32)
        dv2[:PER] = dinv2[PER * i:PER * (i + 1)]
        in_maps.append({
            "emb_own": eo,
            "dinv_col": dv.reshape(NB, BLK).T.copy(),
            "dinv2_col": dv2.reshape(NB, BLK).T.copy(),
            "eidx": eidx_arrs[i],
            "edloc": edloc_arrs[i],
            "fidx": fidx_arrs[i],
            "wt": np.ascontiguousarray(W.T).astype(ml_dtypes.bfloat16),
            "bb": np.broadcast_to(b, (128, 4 * D)).astype(np.float32).copy(),
        })

    meta = {
        "T_cb": T_cb, "ctile_off": ctile_off.astype(int),
        "len_c": [int(t) * BLK for t in tt_c],
        "TT": TT, "TOTLEN": TOTLEN, "OUT_T_c": OUT_T_c.astype(int),
        "OUT_T": OUT_T, "OUTLEN": OUTLEN,
    }
    return in_maps, meta, pos_arrs


def _build(cfg, meta, single=False, repeat=1):
    SEG, NB, NTOT, CHUNK, GCALL, SLAB = (cfg.SEG, cfg.NB, cfg.NTOT, cfg.CHUNK,
                                         cfg.GCALL, cfg.SLAB)
    T_cb = meta["T_cb"]
    ctile_off = meta["ctile_off"]
    len_c = meta["len_c"]
    TT = meta["TT"]
    TOTLEN = meta["TOTLEN"]
    OUT_T_c = meta["OUT_T_c"]
    OUT_T = meta["OUT_T"]
    OUTLEN = meta["OUTLEN"]
    off16 = np.concatenate([[0], np.cumsum([l // 16 for l in len_c])[:-1]]).astype(int)
    foff16 = np.concatenate(
        [[0], np.cumsum([int(t) * BLK // 16 for t in OUT_T_c])[:-1]]).astype(int)
    fbase_t = np.concatenate([[0], np.cumsum(OUT_T_c)[:-1]]).astype(int)

    nc = bacc.Bacc("TRN2", target_bir_lowering=False, debug=False,
                   enable_asserts=False, num_devices=(1 if single else NC))

    def allgather(l):
        if single:
            # timing-only stand-in: copy the own slice into the full table
            nc.sync.dma_start(xs_full[l][:SEG, :], xs_own[l][:])
        else:
            nc.gpsimd.collective_compute(
                "AllGather", mybir.AluOpType.bypass, replica_groups=RG,
                ins=[xs_own[l][:]], outs=[xs_full[l][:]])

    emb_own = nc.dram_tensor("emb_own", [SEG, D], F32, kind="ExternalInput")
    dinv_col = nc.dram_tensor("dinv_col", [128, NB], F32, kind="ExternalInput")
    dinv2_col = nc.dram_tensor("dinv2_col", [128, NB], F32, kind="ExternalInput")
    eidx = nc.dram_tensor("eidx", [128, TOTLEN // 16], I16, kind="ExternalInput")
    edloc = nc.dram_tensor("edloc", [128, TT], F32, kind="ExternalInput")
    fidx = nc.dram_tensor("fidx", [128, OUTLEN // 16], I16, kind="ExternalInput")
    wt = nc.dram_tensor("wt", [4 * D, 4 * D], BF16, kind="ExternalInput")
    bb = nc.dram_tensor("bb", [128, 4 * D], F32, kind="ExternalInput")
    y = nc.dram_tensor("y", [OUTLEN, 4 * D], F32, kind="ExternalOutput")

    xs_own = [nc.dram_tensor(f"xs_own{l}", [SEG, D], BF16) for l in range(NL + 1)]
    xs_full = [nc.dram_tensor(f"xs_full{l}", [NTOT, D], BF16, addr_space="Shared")
               for l in range(NL + 1)]
    RG = [list(range(NC))]

    with tile.TileContext(nc) as tc:
        with tc.tile_pool(name="const", bufs=1) as cpool:
            eidx_sb = cpool.tile([128, TOTLEN // 16], I16, tag="eidx")
            nc.sync.dma_start(eidx_sb[:], eidx[:])
            edloc_sb = cpool.tile([128, TT], F32, tag="edloc")
            nc.sync.dma_start(edloc_sb[:], edloc[:])
            fidx_sb = cpool.tile([128, OUTLEN // 16], I16, tag="fidx")
            nc.sync.dma_start(fidx_sb[:], fidx[:])
            dinv_sb = cpool.tile([128, NB], F32, tag="dinv")
            nc.sync.dma_start(dinv_sb[:], dinv_col[:])
            dinv2_sb = cpool.tile([128, NB], F32, tag="dinv2")
            nc.sync.dma_start(dinv2_sb[:], dinv2_col[:])
            wt_sb = cpool.tile([128, 4, 4 * D], BF16, tag="wt")
            nc.sync.dma_start(wt_sb[:], wt[:].rearrange("(l k) o -> k l o", k=128))
            bb_sb = cpool.tile([128, 4 * D], F32, tag="bb")
            nc.sync.dma_start(bb_sb[:], bb[:])

            iota_i = cpool.tile([128, 128], I32, tag="iota_i")
            nc.gpsimd.iota(iota_i[:], pattern=[[1, 128]], base=0,
                           channel_multiplier=0)
            iota_bf = cpool.tile([128, 128], BF16, tag="iota_bf")
            nc.vector.tensor_copy(iota_bf[:], iota_i[:])
            pidx_i = cpool.tile([128, 1], I32, tag="pidx_i")
            nc.gpsimd.iota(pidx_i[:], pattern=[[0, 1]], base=0,
                           channel_multiplier=1)
            pidx_f = cpool.tile([128, 1], F32, tag="pidx_f")
            nc.vector.tensor_copy(pidx_f[:], pidx_i[:])
            ident = cpool.tile([128, 128], BF16, tag="ident")
            nc.vector.tensor_scalar(ident[:], iota_bf[:], pidx_f[:], None,
                                    mybir.AluOpType.is_equal)

            for _rep in range(repeat):
              with (
                tc.tile_pool(name="gath", bufs=3) as gpool,
                tc.tile_pool(name="sone", bufs=4) as spool,
                tc.tile_pool(name="stag", bufs=2) as stpool,
                tc.tile_pool(name="eps", bufs=4, space="PSUM") as ppool,
              ):
                # ---- xs_0 = dinv * emb -------------------------------------
                embv = emb_own[:].rearrange("(s p) d -> p s d", p=128)
                xs0v = xs_own[0][:].rearrange("(s p) d -> p s d", p=128)
                for s0 in range(0, NB, SLAB):
                    n = min(SLAB, NB - s0)
                    slab = stpool.tile([128, SLAB, D], F32, tag="emb_slab")
                    nc.sync.dma_start(slab[:, :n, :], embv[:, s0:s0 + n, :])
                    stg = stpool.tile([128, SLAB, D], BF16, tag="stg0")
                    for j in range(n):
                        nc.vector.tensor_scalar(
                            stg[:, j, :], slab[:, j, :],
                            dinv_sb[:, s0 + j:s0 + j + 1], None,
                            mybir.AluOpType.mult)
                    nc.sync.dma_start(xs0v[:, s0:s0 + n, :], stg[:, :n, :])
                allgather(0)

                # ---- message-passing layers -------------------------------
                for l in range(NL):
                    windows = [xs_full[l][c * CHUNK:(c + 1) * CHUNK, :]
                               for c in range(NCH)]
                    xsov = xs_own[l + 1][:].rearrange("(s p) d -> p s d", p=128)
                    gtiles = {}
                    next_call = [0] * NCH

                    def ensure(c, tile_hi, gtiles=gtiles, next_call=next_call,
                               windows=windows):
                        while next_call[c] * (GCALL // BLK) < tile_hi:
                            k = next_call[c]
                            n_idx = min(GCALL, len_c[c] - k * GCALL)
                            gt = gpool.tile([128, GCALL // BLK, D], BF16,
                                            tag=f"g{c}")
                            nc.gpsimd.dma_gather(
                                gt[:, :n_idx // BLK, :], windows[c],
                                eidx_sb[:, off16[c] + k * (GCALL // 16):
                                        off16[c] + k * (GCALL // 16) + n_idx // 16],
                                num_idxs=n_idx, num_idxs_reg=n_idx,
                                elem_size=D, single_packet=(n_idx <= 1024))
                            gtiles[(c, k)] = gt
                            next_call[c] += 1

                    pos = [0] * NCH
                    stg = None
                    for b in range(NB):
                        if b % SLAB == 0:
                            stg = stpool.tile([128, SLAB, D], BF16, tag="stgL")
                        nmm = int(sum(T_cb[c][b] for c in range(NCH)))
                        ps = ppool.tile([128, D], F32, tag="ps")
                        mm = 0
                        for c in range(NCH):
                            ensure(c, pos[c] + int(T_cb[c][b]))
                            for t in range(int(T_cb[c][b])):
                                tg = pos[c] + t
                                k, s = divmod(tg, GCALL // BLK)
                                S = spool.tile([128, 128], BF16, tag="S")
                                col = int(ctile_off[c]) + tg
                                nc.vector.tensor_scalar(
                                    S[:], iota_bf[:],
                                    edloc_sb[:, col:col + 1], None,
                                    mybir.AluOpType.is_equal)
                                nc.tensor.matmul(
                                    ps[:], lhsT=S[:],
                                    rhs=gtiles[(c, k)][:, s, :],
                                    start=(mm == 0), stop=(mm == nmm - 1))
                                mm += 1
                            pos[c] += int(T_cb[c][b])
                        nc.scalar.mul(stg[:, b % SLAB, :], ps[:],
                                      dinv2_sb[:, b:b + 1])
                        if b % SLAB == SLAB - 1 or b == NB - 1:
                            s0 = b - b % SLAB
                            nc.sync.dma_start(xsov[:, s0:b + 1, :],
                                              stg[:, :b - s0 + 1, :])
                    allgather(l + 1)

              # ---- final: gather + normalize + concat + MLP + select --------
              with (
                tc.tile_pool(name="fg", bufs=1) as fpool,
                tc.tile_pool(name="fz", bufs=3) as zpool,
                tc.tile_pool(name="fpt", bufs=2, space="PSUM") as ptpool,
                tc.tile_pool(name="fpy", bufs=2, space="PSUM") as pypool,
              ):
                fg = fpool.tile([128, NL + 1, OUT_T, D], BF16, tag="fg")
                for l in range(NL + 1):
                    for c in range(NCH):
                        n_idx = int(OUT_T_c[c]) * BLK
                        nc.gpsimd.dma_gather(
                            fg[:, l, fbase_t[c]:fbase_t[c] + int(OUT_T_c[c]), :],
                            xs_full[l][c * CHUNK:(c + 1) * CHUNK, :],
                            fidx_sb[:, foff16[c]:foff16[c] + n_idx // 16],
                            num_idxs=n_idx, num_idxs_reg=n_idx, elem_size=D,
                            single_packet=(n_idx <= 1024))

                for ot in range(OUT_T):
                    scr = zpool.tile([128, 128], F32, tag="scr")
                    ssq = zpool.tile([128, NL + 1], F32, tag="ssq")
                    for l in range(NL + 1):
                        nc.scalar.activation(
                            scr[:], fg[:, l, ot, :],
                            mybir.ActivationFunctionType.Square,
                            accum_out=ssq[:, l:l + 1])
                    nrm = zpool.tile([128, NL + 1], F32, tag="nrm")
                    nc.scalar.sqrt(nrm[:], ssq[:])
                    nc.vector.tensor_scalar_max(nrm[:], nrm[:], 1e-12)
                    rinv = zpool.tile([128, NL + 1], F32, tag="rinv")
                    nc.vector.reciprocal(rinv[:], nrm[:])
                    zT = zpool.tile([128, NL + 1, 128], BF16, tag="zT")
                    for l in range(NL + 1):
                        z = zpool.tile([128, 128], BF16, tag="z")
                        nc.vector.tensor_scalar(
                            z[:], fg[:, l, ot, :], rinv[:, l:l + 1], None,
                            mybir.AluOpType.mult)
                        pt = ptpool.tile([128, 128], BF16, tag="pt")
                        nc.tensor.transpose(pt[:], z[:], ident[:])
                        nc.scalar.copy(zT[:, l, :], pt[:])
                    py = pypool.tile([128, 4 * D], F32, tag="py")
                    for l in range(NL + 1):
                        nc.tensor.matmul(py[:], lhsT=zT[:, l, :],
                                         rhs=wt_sb[:, l, :],
                                         start=(l == 0), stop=(l == NL))
                    ysb = zpool.tile([128, 4 * D], F32, tag="ysb")
                    nc.vector.tensor_add(ysb[:], py[:], bb_sb[:])
                    nc.sync.dma_start(y[ot * 128:(ot + 1) * 128, :], ysb[:])

    nc.compile()
    return nc


def _run(inputs, trace=False, cfg=FULL):
    from concourse.bass_utils import run_bass_kernel_spmd

    emb = np.asarray(inputs["emb"], np.float32)
    edge_index = np.asarray(inputs["edge_index"])
    senders = np.asarray(inputs["senders"])
    receivers = np.asarray(inputs["receivers"])
    W = np.asarray(inputs["W"], np.float32)
    b = np.asarray(inputs["b"], np.float32)

    in_maps, meta, pos_arrs = _prep(cfg, emb, edge_index, senders, receivers, W, b)
    nc = _build(cfg, meta)
    res = run_bass_kernel_spmd(nc, in_maps, list(range(NC)), trace=trace)

    s_out = np.empty((cfg.NOUT, 4 * D), np.float32)
    r_out = np.empty((cfg.NOUT, 4 * D), np.float32)
    OPC = cfg.OPC
    for i in range(NC):
        yv = res.results[i]["y"]
        s_out[OPC * i:OPC * (i + 1)] = yv[pos_arrs[i][:OPC]]
        r_out[OPC * i:OPC * (i + 1)] = yv[pos_arrs[i][OPC:]]
    return (s_out, r_out), res


def kernel(**inputs):
    out, _ = _run(inputs, trace=False)
    return out



# revision 12
# speedup vs baseline: 1.5806x; 1.5806x over previous
"""LightGCN-style GNN message passing on 8 Trainium2 NeuronCores (v2).

Algorithm (matches the reference):
    deg  = bincount(dst);  dinv = rsqrt(max(deg, 1))
    x_{l+1} = dinv * (A @ (dinv * x_l))          (3 layers, A = binary adjacency)
    z_l = l2_normalize(x_l);  Z = concat(z_0..z_3);  Y = Z @ W.T + b
    return Y[senders], Y[receivers]

Factorization: with xs_l = dinv * x_l, messages need no per-edge scale and
l2_normalize(xs_l) == l2_normalize(x_l); only xs tables are materialized (bf16).

Sharding: destination-sharded.  Core i owns N/8 dst rows.  The dst rows are
split into NP=4 pieces; the AllGather of each xs table is issued per piece as
soon as that piece's rows are computed, so collectives overlap the remaining
blocks' compute and the next layer's gathers (which only wait on the piece
they read from).

Edge schedule: per core, 4 gather streams (one per src piece).  Within a
stream, edges are grouped by dst block with cells padded to 16-index
granularity only (L_pb = max-over-cores, >=128).  Fixed 128-slot matmul
windows run over each stream; a window that straddles a cell boundary issues
two one-hot matmuls (S built on DVE via iota + is_equal; -1 entries match
nothing), accumulating the segment-sum in PSUM per dst block on PE.
This cuts gather descriptors (the measured bottleneck: ~7.7ns/descriptor,
descriptor-count-bound) from 25% padding overhead to ~10%.

The host only computes degrees and integer index/schedule arrays; all FLOAT
work on emb/W/b happens on device.
"""

import numpy as np
import ml_dtypes

import concourse.bacc as bacc
import concourse.mybir as mybir
import concourse.tile as tile

F32 = mybir.dt.float32
BF16 = mybir.dt.bfloat16
I16 = mybir.dt.int16
I32 = mybir.dt.int32

D = 128             # feature dim
NL = 3              # message passing layers
NC = 8              # cores
BLK = 128           # dst block (psum partition dim)
NP = 4              # src/dst pieces (gather windows + AllGather pipeline)


def _ceil(a, b):
    return (a + b - 1) // b


class Cfg:
    def __init__(self, N, E, NOUT, GCALL=4096):
        self.N = N
        self.E = E
        self.NOUT = NOUT
        self.GCALL = GCALL
        self.PER = N // NC
        self.NB = _ceil(self.PER, BLK)
        self.SEG = self.NB * BLK
        base, rem = divmod(self.NB, NP)
        self.PB = [base + (1 if q < rem else 0) for q in range(NP)]
        self.PSTART = np.concatenate([[0], np.cumsum(self.PB)[:-1]]).astype(int)
        self.PR = [pb * BLK for pb in self.PB]          # rows/piece/core
        self.WROWS = [NC * pr for pr in self.PR]        # gather window rows
        assert max(self.WROWS) <= 32767, "int16 gather index overflow"
        self.OPC = NOUT // NC


FULL = Cfg(N=100000, E=1600000, NOUT=16384)


def _wrap16(idx):
    """int16 [L] -> [128, L//16] wrapped in 16 partitions, replicated x8."""
    return np.tile(idx.reshape(-1, 16).T, (8, 1)).copy()


def _prep(cfg, emb, edge_index, senders, receivers, W, b):
    N, E, PER, SEG, NB = cfg.N, cfg.E, cfg.PER, cfg.SEG, cfg.NB
    PB, PSTART, PR = cfg.PB, np.asarray(cfg.PSTART), cfg.PR
    src = np.asarray(edge_index[0], np.int64)
    dst = np.asarray(edge_index[1], np.int64)
    senders = np.asarray(senders, np.int64)
    receivers = np.asarray(receivers, np.int64)
    bias = np.asarray(b, np.float32)    # `b` is shadowed by loop vars below

    deg = np.bincount(dst, minlength=N).astype(np.float32)
    deg = np.maximum(deg, 1.0)
    dinv = (1.0 / np.sqrt(deg)).astype(np.float32)
    dinv2 = (dinv * dinv).astype(np.float32)

    piece_of_block = np.zeros(NB, np.int64)
    for q in range(NP):
        piece_of_block[PSTART[q]:PSTART[q] + PB[q]] = q
    PRa = np.asarray(PR)

    def node_piece_idx(x):
        """node id -> (piece, in-window row)"""
        ci = x // PER
        r = x % PER
        blk = r // BLK
        p = piece_of_block[blk]
        sidx = ci * PRa[p] + (blk - PSTART[p]) * BLK + (r % BLK)
        return p, sidx

    p_s, sidx = node_piece_idx(src)
    ci_d = dst // PER
    r_d = dst % PER
    b_d = r_d // BLK
    dloc = r_d % BLK

    # --- cell sizes & stream layout (shared across cores) ------------------
    key = (ci_d * NP + p_s) * NB + b_d
    counts = np.bincount(key, minlength=NC * NP * NB).reshape(NC, NP, NB)
    L_pb = np.maximum(_ceil(counts.max(axis=0), 16) * 16, BLK)     # [NP, NB]
    O_pb = np.zeros((NP, NB), np.int64)                            # cell base
    stream_len = np.zeros(NP, np.int64)
    for p in range(NP):
        O_pb[p] = np.concatenate([[0], np.cumsum(L_pb[p])[:-1]])
        stream_len[p] = _ceil(int(L_pb[p].sum()), BLK) * BLK
    W_p = (stream_len // BLK).astype(int)
    W_off = np.concatenate([[0], np.cumsum(W_p)[:-1]]).astype(int)
    Wtot = int(W_p.sum())
    stream_base = np.concatenate([[0], np.cumsum(stream_len)[:-1]]).astype(int)
    TOTLEN = int(stream_len.sum())
    off16 = (stream_base // 16).astype(int)

    # window -> start block / straddle (shared)
    bstart_w = np.zeros(Wtot, np.int64)
    straddle_w = np.zeros(Wtot, bool)
    for p in range(NP):
        ends = np.cumsum(L_pb[p])                  # cell end positions
        L_real = int(L_pb[p].sum())
        for wl in range(W_p[p]):
            s0 = wl * BLK
            if s0 >= L_real:
                bstart_w[W_off[p] + wl] = NB       # pure-pad tail window
                continue
            b0 = int(np.searchsorted(ends, s0, side="right"))
            send = min(s0 + BLK - 1, L_real - 1)
            b1 = int(np.searchsorted(ends, send, side="right"))
            assert b1 - b0 <= 1, "window spans >2 cells"
            bstart_w[W_off[p] + wl] = b0
            straddle_w[W_off[p] + wl] = b1 != b0

    # schedule: per phase (dst block) the matmul ops in program order
    # op = (p, w_global, which, start, stop); target psum = b + which
    sched = [[] for _ in range(NB)]
    for b in range(NB):
        for p in range(NP):
            wg = W_off[p] + np.nonzero(bstart_w[W_off[p]:W_off[p] + W_p[p]] == b)[0]
            for w in wg:
                sched[b].append([p, int(w), 0, False, False])
                if straddle_w[w]:
                    sched[b].append([p, int(w), 1, False, False])
    first = {}
    last = {}
    for b in range(NB):
        for oi, op in enumerate(sched[b]):
            tgt = b + op[2]
            if tgt not in first:
                first[tgt] = (b, oi)
            last[tgt] = (b, oi)
    for tgt, (b, oi) in first.items():
        sched[b][oi][3] = True
    for tgt, (b, oi) in last.items():
        sched[b][oi][4] = True
    assert set(first) == set(range(NB))

    # --- per-core edge index / edloc arrays --------------------------------
    order = np.argsort(key, kind="stable")
    cnt_flat = counts.reshape(-1)
    starts_flat = np.concatenate([[0], np.cumsum(cnt_flat)[:-1]])
    rank = np.arange(E, dtype=np.int64) - starts_flat[key[order]]
    # position within the core's piece stream
    p_o = p_s[order]
    b_o = b_d[order]
    pos = O_pb[p_o, b_o] + rank                    # in-stream position
    core_o = ci_d[order]
    sidx_o = sidx[order]
    dloc_o = dloc[order]

    eidx_arrs, edloc_arrs = [], []
    for i in range(NC):
        m = core_o == i
        ia = np.zeros(TOTLEN, np.int16)
        ia[stream_base[p_o[m]] + pos[m]] = sidx_o[m].astype(np.int16)
        ed = np.full((BLK, 2 * Wtot), -1.0, np.float32)
        wg = W_off[p_o[m]] + pos[m] // BLK
        j = pos[m] % BLK
        which = (b_o[m] != bstart_w[wg]).astype(np.int64)
        ed[j, 2 * wg + which] = dloc_o[m]
        eidx_arrs.append(_wrap16(ia))
        edloc_arrs.append(ed)

    # --- output-row schedule ----------------------------------------------
    OPC = cfg.OPC
    ids = [np.concatenate([senders[OPC * i:OPC * (i + 1)],
                           receivers[OPC * i:OPC * (i + 1)]]) for i in range(NC)]
    fp = [node_piece_idx(x) for x in ids]
    gcnt = np.array([[int((fp[i][0] == q).sum()) for q in range(NP)]
                     for i in range(NC)])
    OUT_T_q = _ceil(gcnt, BLK).max(axis=0)         # [NP] tiles, shared
    OUT_T = int(OUT_T_q.sum())
    OUTLEN = OUT_T * BLK
    fbase = np.concatenate([[0], np.cumsum(OUT_T_q)[:-1]]) * BLK

    fidx_arrs, pos_arrs = [], []
    for i in range(NC):
        fq, fsi = fp[i]
        fo = np.argsort(fq, kind="stable")
        posi = np.zeros(2 * OPC, np.int64)
        ia = np.zeros(OUTLEN, np.int16)
        for q in range(NP):
            mm = fo[fq[fo] == q]
            pp = fbase[q] + np.arange(len(mm))
            posi[mm] = pp
            ia[pp] = fsi[mm].astype(np.int16)
        fidx_arrs.append(_wrap16(ia))
        pos_arrs.append(posi)

    # --- per-core dense inputs --------------------------------------------
    in_maps = []
    for i in range(NC):
        eo = np.zeros((SEG, D), np.float32)
        eo[:PER] = emb[PER * i:PER * (i + 1)]
        dv = np.zeros(SEG, np.float32)
        dv[:PER] = dinv[PER * i:PER * (i + 1)]
        dv2 = np.zeros(SEG, np.float32)
        dv2[:PER] = dinv2[PER * i:PER * (i + 1)]
        in_maps.append({
            "emb_own": eo,
            "dinv_col": dv.reshape(NB, BLK).T.copy(),
            "dinv2_col": dv2.reshape(NB, BLK).T.copy(),
            "eidx": eidx_arrs[i],
            "edloc": edloc_arrs[i],
            "fidx": fidx_arrs[i],
            "wt": np.ascontiguousarray(W.T).astype(ml_dtypes.bfloat16),
            "bb": np.broadcast_to(bias, (BLK, 4 * D)).astype(np.float32).copy(),
        })

    meta = {
        "sched": sched, "W_p": W_p, "W_off": W_off, "Wtot": Wtot,
        "stream_len": stream_len.astype(int), "off16": off16,
        "TOTLEN": TOTLEN,
        "OUT_T_q": OUT_T_q.astype(int), "OUT_T": OUT_T, "OUTLEN": OUTLEN,
    }
    return in_maps, meta, pos_arrs


def _build(cfg, meta, single=False, repeat=1, dbg=0):
    SEG, NB, GCALL = cfg.SEG, cfg.NB, cfg.GCALL
    PB, PSTART, WROWS = cfg.PB, cfg.PSTART, cfg.WROWS
    sched = meta["sched"]
    W_p = meta["W_p"]
    W_off = meta["W_off"]
    Wtot = meta["Wtot"]
    stream_len = meta["stream_len"]
    off16 = meta["off16"]
    TOTLEN = meta["TOTLEN"]
    OUT_T_q = meta["OUT_T_q"]
    OUT_T = meta["OUT_T"]
    OUTLEN = meta["OUTLEN"]
    CPB = GCALL // BLK                              # windows per gather call
    ncall_p = [_ceil(int(stream_len[p]), GCALL) for p in range(NP)]
    foff16 = np.concatenate(
        [[0], np.cumsum([int(t) * BLK // 16 for t in OUT_T_q])[:-1]]).astype(int)
    fbase_t = np.concatenate([[0], np.cumsum(OUT_T_q)[:-1]]).astype(int)
    MAXSB = _ceil(max(PB), 2)                       # sub-slab blocks (SBUF)

    nc = bacc.Bacc("TRN2", target_bir_lowering=False, debug=False,
                   enable_asserts=False, num_devices=(1 if single else NC))

    emb_own = nc.dram_tensor("emb_own", [SEG, D], F32, kind="ExternalInput")
    dinv_col = nc.dram_tensor("dinv_col", [128, NB], F32, kind="ExternalInput")
    dinv2_col = nc.dram_tensor("dinv2_col", [128, NB], F32, kind="ExternalInput")
    eidx = nc.dram_tensor("eidx", [128, TOTLEN // 16], I16, kind="ExternalInput")
    edloc = nc.dram_tensor("edloc", [128, 2 * Wtot], F32, kind="ExternalInput")
    fidx = nc.dram_tensor("fidx", [128, OUTLEN // 16], I16, kind="ExternalInput")
    wt = nc.dram_tensor("wt", [4 * D, 4 * D], BF16, kind="ExternalInput")
    bb = nc.dram_tensor("bb", [128, 4 * D], F32, kind="ExternalInput")
    y = nc.dram_tensor("y", [OUTLEN, 4 * D], F32, kind="ExternalOutput")

    xs_own = [nc.dram_tensor(f"xs_own{l}", [SEG, D], BF16) for l in range(NL + 1)]
    xs_piece = [[nc.dram_tensor(f"xs_p{l}_{q}", [WROWS[q], D], BF16,
                                addr_space="Shared") for q in range(NP)]
                for l in range(NL + 1)]
    RG = [list(range(NC))]

    def allgather(l, q):
        rows = slice(PSTART[q] * BLK, (PSTART[q] + PB[q]) * BLK)
        if single:
            nc.sync.dma_start(xs_piece[l][q][:PB[q] * BLK, :], xs_own[l][rows, :])
        else:
            nc.gpsimd.collective_compute(
                "AllGather", mybir.AluOpType.bypass, replica_groups=RG,
                ins=[xs_own[l][rows, :]], outs=[xs_piece[l][q][:]])

    with tile.TileContext(nc) as tc:
        with tc.tile_pool(name="const", bufs=1) as cpool:
            dinv_sb = cpool.tile([128, NB], F32, tag="dinv")
            nc.sync.dma_start(dinv_sb[:], dinv_col[:])
            dinv2_sb = cpool.tile([128, NB], F32, tag="dinv2")
            nc.sync.dma_start(dinv2_sb[:], dinv2_col[:])
            eidx_sb = cpool.tile([128, TOTLEN // 16], I16, tag="eidx")
            nc.sync.dma_start(eidx_sb[:], eidx[:])
            edloc_sb = cpool.tile([128, 2 * Wtot], F32, tag="edloc")
            nc.sync.dma_start(edloc_sb[:], edloc[:])
            fidx_sb = cpool.tile([128, OUTLEN // 16], I16, tag="fidx")
            nc.sync.dma_start(fidx_sb[:], fidx[:])
            wt_sb = cpool.tile([128, 4, 4 * D], BF16, tag="wt")
            nc.sync.dma_start(wt_sb[:], wt[:].rearrange("(l k) o -> k l o", k=128))
            bb_sb = cpool.tile([128, 4 * D], F32, tag="bb")
            nc.sync.dma_start(bb_sb[:], bb[:])

            iota_i = cpool.tile([128, 128], I32, tag="iota_i")
            nc.gpsimd.iota(iota_i[:], pattern=[[1, 128]], base=0,
                           channel_multiplier=0)
            iota_bf = cpool.tile([128, 128], BF16, tag="iota_bf")
            nc.vector.tensor_copy(iota_bf[:], iota_i[:])
            pidx_i = cpool.tile([128, 1], I32, tag="pidx_i")
            nc.gpsimd.iota(pidx_i[:], pattern=[[0, 1]], base=0,
                           channel_multiplier=1)
            pidx_f = cpool.tile([128, 1], F32, tag="pidx_f")
            nc.vector.tensor_copy(pidx_f[:], pidx_i[:])
            ident = cpool.tile([128, 128], BF16, tag="ident")
            nc.vector.tensor_scalar(ident[:], iota_bf[:], pidx_f[:], None,
                                    mybir.AluOpType.is_equal)

            fg = cpool.tile([128, NL + 1, OUT_T, D], BF16, tag="fg")

            def fgather(l):
                for q in range(NP):
                    n_idx = int(OUT_T_q[q]) * BLK
                    nc.gpsimd.dma_gather(
                        fg[:, l, fbase_t[q]:fbase_t[q] + int(OUT_T_q[q]), :],
                        xs_piece[l][q][:],
                        fidx_sb[:, foff16[q]:foff16[q] + n_idx // 16],
                        num_idxs=n_idx, num_idxs_reg=n_idx, elem_size=D,
                        single_packet=(n_idx <= 1024))

            for _rep in range(repeat):
              with (
                tc.tile_pool(name="gath", bufs=2) as gpool,
                tc.tile_pool(name="sone", bufs=8) as spool,
                tc.tile_pool(name="stag", bufs=2) as stpool,
                tc.tile_pool(name="eps", bufs=4, space="PSUM") as ppool,
              ):
                # ---- xs_0 = dinv * emb, piecewise + AG pipeline ------------
                embv = emb_own[:].rearrange("(s p) d -> p s d", p=128)
                xs0v = xs_own[0][:].rearrange("(s p) d -> p s d", p=128)
                for q in range(NP):
                    s0, n = int(PSTART[q]), PB[q]
                    for t0 in range(0, n, MAXSB):
                        tn = min(MAXSB, n - t0)
                        slab = stpool.tile([128, MAXSB, D], F32, tag="emb_slab")
                        nc.sync.dma_start(slab[:, :tn, :],
                                          embv[:, s0 + t0:s0 + t0 + tn, :])
                        stg = stpool.tile([128, MAXSB, D], BF16, tag="stg0")
                        for j in range(tn):
                            nc.vector.tensor_scalar(
                                stg[:, j, :], slab[:, j, :],
                                dinv_sb[:, s0 + t0 + j:s0 + t0 + j + 1], None,
                                mybir.AluOpType.mult)
                        nc.sync.dma_start(xs0v[:, s0 + t0:s0 + t0 + tn, :],
                                          stg[:, :tn, :])
                    allgather(0, q)

                # ---- message-passing layers -------------------------------
                for l in range(NL):
                    windows = [xs_piece[l][q][:] for q in range(NP)]
                    xsov = xs_own[l + 1][:].rearrange("(s p) d -> p s d", p=128)
                    gtiles = {}
                    next_call = [0] * NP

                    def ensure(p, w_hi, gtiles=gtiles, next_call=next_call,
                               windows=windows):
                        # issue gather calls covering windows [0, w_hi) of
                        # stream p
                        while next_call[p] * CPB < w_hi:
                            k = next_call[p]
                            n_idx = min(GCALL, int(stream_len[p]) - k * GCALL)
                            gt = gpool.tile([128, CPB, D], BF16, tag=f"g{p}")
                            nc.gpsimd.dma_gather(
                                gt[:, :n_idx // BLK, :], windows[p],
                                eidx_sb[:, off16[p] + k * (GCALL // 16):
                                        off16[p] + k * (GCALL // 16) + n_idx // 16],
                                num_idxs=n_idx, num_idxs_reg=n_idx,
                                elem_size=D, single_packet=(n_idx <= 1024))
                            gtiles[(p, k)] = gt
                            next_call[p] += 1

                    psums = {}
                    stg = None
                    sub0 = 0
                    qcur = 0
                    for b in range(NB):
                        if (b - int(PSTART[qcur])) % MAXSB == 0:
                            stg = stpool.tile([128, MAXSB, D], BF16, tag="stgL")
                            sub0 = b
                        # prefetch gathers for all windows used in this phase
                        for p in range(NP):
                            mx = 0
                            for op in sched[b]:
                                if op[0] == p:
                                    mx = max(mx, op[1] - W_off[p] + 1)
                            if mx:
                                ensure(p, mx)
                        for p_, w_, which, st, sp in sched[b]:
                            wl = w_ - W_off[p_]
                            k, s = divmod(wl, CPB)
                            S = spool.tile([128, 128], BF16, tag="S")
                            col = 2 * w_ + which
                            nc.vector.tensor_scalar(
                                S[:], iota_bf[:],
                                edloc_sb[:, col:col + 1], None,
                                mybir.AluOpType.is_equal)
                            tgt = b + which
                            if st:
                                ps_new = ppool.tile([128, D], F32, tag="ps")
                                psums[tgt] = ps_new
                            nc.tensor.matmul(
                                psums[tgt][:], lhsT=S[:],
                                rhs=gtiles[(p_, k)][:, s, :],
                                start=st, stop=sp)
                        ps = psums.pop(b)
                        nc.scalar.mul(stg[:, b - sub0, :], ps[:],
                                      dinv2_sb[:, b:b + 1])
                        pend = int(PSTART[qcur]) + PB[qcur] - 1
                        if b == pend or b - sub0 == MAXSB - 1:
                            nc.sync.dma_start(xsov[:, sub0:b + 1, :],
                                              stg[:, :b - sub0 + 1, :])
                            if b == pend:
                                allgather(l + 1, qcur)
                                qcur = min(qcur + 1, NP - 1)
                    assert not psums
                    # prefetch final gathers of table l (AG(l) done by now)
                    if l >= 1:
                        fgather(l - 1)

              # ---- final: gather + normalize + concat + MLP + select -------
              fgather(NL - 1)
              fgather(NL)
              with (
                tc.tile_pool(name="fz", bufs=3) as zpool,
                tc.tile_pool(name="fpt", bufs=2, space="PSUM") as ptpool,
                tc.tile_pool(name="fpy", bufs=2, space="PSUM") as pypool,
              ):
                for ot in range(OUT_T):
                    scr = zpool.tile([128, 128], F32, tag="scr")
                    ssq = zpool.tile([128, NL + 1], F32, tag="ssq")
                    for l in range(NL + 1):
                        nc.scalar.activation(
                            scr[:], fg[:, l, ot, :],
                            mybir.ActivationFunctionType.Square,
                            accum_out=ssq[:, l:l + 1])
                    nrm = zpool.tile([128, NL + 1], F32, tag="nrm")
                    nc.scalar.sqrt(nrm[:], ssq[:])
                    nc.vector.tensor_scalar_max(nrm[:], nrm[:], 1e-12)
                    rinv = zpool.tile([128, NL + 1], F32, tag="rinv")
                    nc.vector.reciprocal(rinv[:], nrm[:])
                    zT = zpool.tile([128, NL + 1, 128], BF16, tag="zT")
                    for l in range(NL + 1):
                        z = zpool.tile([128, 128], BF16, tag="z")
                        nc.vector.tensor_scalar(
                            z[:], fg[:, l, ot, :], rinv[:, l:l + 1], None,
                            mybir.AluOpType.mult)
                        pt = ptpool.tile([128, 128], BF16, tag="pt")
                        nc.tensor.transpose(pt[:], z[:], ident[:])
                        nc.scalar.copy(zT[:, l, :], pt[:])
                    py = pypool.tile([128, 4 * D], F32, tag="py")
                    for l in range(NL + 1):
                        nc.tensor.matmul(py[:], lhsT=zT[:, l, :],
                                         rhs=wt_sb[:, l, :],
                                         start=(l == 0), stop=(l == NL))
                    ysb = zpool.tile([128, 4 * D], F32, tag="ysb")
                    if dbg == 1:        # skip bias: isolate py
                        nc.scalar.copy(ysb[:], py[:])
                    elif dbg == 2:      # bias only: isolate bb_sb
                        nc.scalar.copy(ysb[:], bb_sb[:])
                    elif dbg == 3:      # fresh load of bb dram
                        bb2 = zpool.tile([128, 4 * D], F32, tag="bb2")
                        nc.sync.dma_start(bb2[:], bb[:])
                        nc.scalar.copy(ysb[:], bb2[:])
                    else:
                        nc.vector.tensor_add(ysb[:], py[:], bb_sb[:])
                    nc.sync.dma_start(y[ot * 128:(ot + 1) * 128, :], ysb[:])

    nc.compile()
    return nc


def _run(inputs, trace=False, cfg=FULL):
    from concourse.bass_utils import run_bass_kernel_spmd

    emb = np.asarray(inputs["emb"], np.float32)
    edge_index = np.asarray(inputs["edge_index"])
    senders = np.asarray(inputs["senders"])
    receivers = np.asarray(inputs["receivers"])
    W = np.asarray(inputs["W"], np.float32)
    b = np.asarray(inputs["b"], np.float32)

    in_maps, meta, pos_arrs = _prep(cfg, emb, edge_index, senders, receivers, W, b)
    nc = _build(cfg, meta)
    res = run_bass_kernel_spmd(nc, in_maps, list(range(NC)), trace=trace)

    s_out = np.empty((cfg.NOUT, 4 * D), np.float32)
    r_out = np.empty((cfg.NOUT, 4 * D), np.float32)
    OPC = cfg.OPC
    for i in range(NC):
        yv = res.results[i]["y"]
        s_out[OPC * i:OPC * (i + 1)] = yv[pos_arrs[i][:OPC]]
        r_out[OPC * i:OPC * (i + 1)] = yv[pos_arrs[i][OPC:]]
    return (s_out, r_out), res


def kernel(**inputs):
    out, _ = _run(inputs, trace=False)
    return out
